# revision 4
# baseline (speedup 1.0000x reference)
"""Trainium2 Bass kernel for nn_LowpassDetector.

Computes: power = re^2 + im^2, 5-tap FIR (b), order-4 IIR recurrence (a)
along time, for signal [2, T=16384, B=2048] -> y [T, B].

Strategy: the FIR+IIR cascade is LTI with all poles at radius <= 0.758,
so the combined impulse response h decays below 1e-15 within 128 taps.
The whole filter is therefore exactly (to fp32) a block-Toeplitz matmul:
  y_blk[b] = T0 @ x_blk[b] + T1 @ x_blk[b-1]     (b >= 1)
  y_blk[0] = L0 @ x_blk[0]
where L0 is the exact 128x128 operator of the reference recurrence
(including its nonstandard "first 5 samples pass through" initial
condition), built on the host in float64 by running the reference on
basis vectors. Channels (2048) are sharded 256 per core across 8 cores;
time blocks of 128 map to the TensorEngine contraction dim.
"""

import sys
from contextlib import ExitStack

import numpy as np

for _p in ("/opt/trn_rl_repo",):
    if _p not in sys.path:
        sys.path.insert(0, _p)

import concourse.bass as bass  # noqa: E402
import concourse.tile as tile  # noqa: E402
from concourse import bacc, mybir  # noqa: E402
from concourse.bass_utils import run_bass_kernel_spmd  # noqa: E402

T, B, NCORES = 16384, 2048, 8
BL = 128                # time-block size (= PE contraction dim)
NB = T // BL            # 128 time blocks
C = B // NCORES         # 256 channels per core
F32 = mybir.dt.float32
F32R = mybir.dt.float32r

USE_F32R = True         # float32r matmul: 4x faster PE, fp32-class accuracy
TRACE = False           # set by test harness for NTFF profiling
LAST_RESULTS = None     # BassKernelResults of the last run (for profiling)

_program_cache = {}


def _reference_operator(bb, aa, n):
    """Exact linear operator of the reference filter on n samples (float64).

    Columns are responses to basis vectors; replicates the reference
    semantics: xf = zero-padded cross-correlation with b, first 5 outputs
    pass through, recurrence y[t] = xf[t] - sum_j a_j y[t-j] from t=5.
    """
    x = np.eye(n)
    xp = np.concatenate([np.zeros((4, n)), x], 0)
    xf = sum(bb[k] * xp[k:k + n] for k in range(5))
    y = xf.copy()
    at = aa[:4]
    for t in range(5, n):
        y[t] = xf[t] - (at[0] * y[t - 4] + at[1] * y[t - 3]
                        + at[2] * y[t - 2] + at[3] * y[t - 1])
    return y


def _build_mats(b32, a32):
    bb = np.asarray(b32, np.float64)
    aa = np.asarray(a32, np.float64)
    M = _reference_operator(bb, aa, 3 * BL)
    L0 = M[0:BL, 0:BL]
    T0 = M[2 * BL:3 * BL, 2 * BL:3 * BL]
    T1 = M[2 * BL:3 * BL, BL:2 * BL]
    # truncation + init-transient leakage must be below fp32 noise
    leak = np.abs(M[2 * BL:3 * BL, 0:BL]).max()
    dev = max(np.abs(M[BL:2 * BL, BL:2 * BL] - T0).max(),
              np.abs(M[BL:2 * BL, 0:BL] - T1).max())
    assert leak < 1e-9 and dev < 1e-9, (leak, dev)
    # matmul computes lhsT.T @ rhs, so ship W.T as the stationary operand
    as_lhsT = lambda W: np.ascontiguousarray(W.T.astype(np.float32))  # noqa: E731
    return as_lhsT(L0), as_lhsT(T0), as_lhsT(T1)


def _build_program():
    nc = bacc.Bacc("TRN2", target_bir_lowering=False, debug=False)
    mm_dt = F32R if USE_F32R else F32
    sig = nc.dram_tensor("sig", [2, T, C], F32, kind="ExternalInput").ap()
    w0d = nc.dram_tensor("w0T", [BL, BL], mm_dt, kind="ExternalInput").ap()
    t0d = nc.dram_tensor("t0T", [BL, BL], mm_dt, kind="ExternalInput").ap()
    t1d = nc.dram_tensor("t1T", [BL, BL], mm_dt, kind="ExternalInput").ap()
    yd = nc.dram_tensor("y", [T, C], F32, kind="ExternalOutput").ap()

    with tile.TileContext(nc) as tc, ExitStack() as ctx:
        wpool = ctx.enter_context(tc.tile_pool(name="w", bufs=1))
        w0 = wpool.tile([BL, BL], mm_dt, tag="w0")
        t0 = wpool.tile([BL, BL], mm_dt, tag="t0")
        t1 = wpool.tile([BL, BL], mm_dt, tag="t1")
        nc.sync.dma_start(w0[:], w0d)
        nc.sync.dma_start(t0[:], t0d)
        nc.sync.dma_start(t1[:], t1d)

        inpool = ctx.enter_context(tc.tile_pool(name="in", bufs=4))
        xpool = ctx.enter_context(tc.tile_pool(name="x", bufs=4))
        ypool = ctx.enter_context(tc.tile_pool(name="y", bufs=4))
        pspool = ctx.enter_context(tc.tile_pool(name="ps", bufs=4, space="PSUM"))

        x_prev = None
        for bl in range(NB):
            rows = bass.ts(bl, BL)
            re = inpool.tile([BL, C], F32, tag="re")
            im = inpool.tile([BL, C], F32, tag="im")
            nc.sync.dma_start(re[:], sig[0, rows, :])
            nc.sync.dma_start(im[:], sig[1, rows, :])

            sq = inpool.tile([BL, C], F32, tag="sq")
            nc.scalar.activation(sq[:], re[:], mybir.ActivationFunctionType.Square)
            sq2 = inpool.tile([BL, C], F32, tag="sq2")
            nc.scalar.activation(sq2[:], im[:], mybir.ActivationFunctionType.Square)
            x = xpool.tile([BL, C], mm_dt, tag="x")
            nc.vector.tensor_add(x[:], sq[:], sq2[:])

            ps = pspool.tile([BL, C], F32)
            if bl == 0:
                nc.tensor.matmul(ps[:], w0[:], x[:], start=True, stop=True)
            else:
                nc.tensor.matmul(ps[:], t0[:], x[:], start=True, stop=False)
                nc.tensor.matmul(ps[:], t1[:], x_prev[:],
                                 start=False, stop=True)

            yt = ypool.tile([BL, C], F32, tag="yt")
            nc.vector.tensor_copy(yt[:], ps[:])
            nc.sync.dma_start(yd[rows, :], yt[:])
            x_prev = x

    nc.compile()
    return nc


def kernel(signal, b, a):
    global LAST_RESULTS
    signal = np.ascontiguousarray(np.asarray(signal), dtype=np.float32)
    assert signal.shape == (2, T, B), signal.shape

    w0T, t0T, t1T = _build_mats(np.asarray(b), np.asarray(a))

    key = "prog"
    if key not in _program_cache:
        _program_cache[key] = _build_program()
    nc = _program_cache[key]

    in_maps = []
    for c in range(NCORES):
        sl = signal[:, :, c * C:(c + 1) * C]
        in_maps.append({
            "sig": np.ascontiguousarray(sl),
            "w0T": w0T, "t0T": t0T, "t1T": t1T,
        })

    res = run_bass_kernel_spmd(nc, in_maps, core_ids=list(range(NCORES)),
                               trace=TRACE)
    LAST_RESULTS = res

    out = np.empty((T, B), np.float32)
    for c in range(NCORES):
        out[:, c * C:(c + 1) * C] = res.results[c]["y"]
    return out


# revision 12
# speedup vs baseline: 1.4574x; 1.4574x over previous
"""Trainium2 Bass kernel for nn_LowpassDetector.

Computes: power = re^2 + im^2, 5-tap FIR (b), order-4 IIR recurrence (a)
along time, for signal [2, T=16384, B=2048] -> y [T, B].

Strategy: the FIR+IIR cascade is LTI with all poles at radius <= 0.758,
so the combined impulse response h decays below 1e-15 within 128 taps.
The whole filter is therefore exactly (to fp32) a block-Toeplitz matmul:
  y_blk[b] = T0 @ x_blk[b] + T1 @ x_blk[b-1]     (b >= 1)
  y_blk[0] = L0 @ x_blk[0]
where L0 is the exact 128x128 operator of the reference recurrence
(including its nonstandard "first 5 samples pass through" initial
condition), built on the host in float64 by running the reference on
basis vectors. Channels (2048) are sharded 256 per core across 8 cores;
time blocks of 128 map to the TensorEngine contraction dim.

Implementation notes (v2):
- Matmuls run in bf16 with an error-compensated 3-term split
  (W ~ Wh + Wl, x ~ xh + xl; y = Wh@xh + Wh@xl + Wl@xh, the dropped
  Wl@xl term is ~2^-16 relative): bf16 streams 1 col/cycle with fast
  weight loads, vs fp32's 4 cycles + slow serial LDWEIGHTS.
- 8 time blocks (1024 steps) are processed per superbatch: single 1MB
  input DMAs and [128, 2048] elementwise tiles amortize the ~600ns
  per-instruction issue cost that dominated v1.
- Two adjacent time blocks share one N=512 matmul (their channel
  columns are adjacent in the x tile free dim), halving matmul count.
- Matmul outputs go PSUM -> DRAM directly by DMA; no SBUF copy.
"""

import sys
from contextlib import ExitStack

import numpy as np
import ml_dtypes

for _p in ("/opt/trn_rl_repo",):
    if _p not in sys.path:
        sys.path.insert(0, _p)

import concourse.bass as bass  # noqa: E402
import concourse.tile as tile  # noqa: E402
from concourse import bacc, mybir  # noqa: E402
from concourse.bass_utils import run_bass_kernel_spmd  # noqa: E402

T, B, NCORES = 16384, 2048, 8
BL = 128                # time-block size (= PE contraction dim)
NB = T // BL            # 128 time blocks
C = B // NCORES         # 256 channels per core
SBW = 8                 # time blocks per superbatch
NSB = NB // SBW         # 16 superbatches
F32 = mybir.dt.float32
BF16 = mybir.dt.bfloat16

TRACE = False           # set by test harness for NTFF profiling
LAST_RESULTS = None     # BassKernelResults of the last run (for profiling)

_program_cache = {}


def _reference_operator(bb, aa, n):
    """Exact linear operator of the reference filter on n samples (float64).

    Columns are responses to basis vectors; replicates the reference
    semantics: xf = zero-padded cross-correlation with b, first 5 outputs
    pass through, recurrence y[t] = xf[t] - sum_j a_j y[t-j] from t=5.
    """
    x = np.eye(n)
    xp = np.concatenate([np.zeros((4, n)), x], 0)
    xf = sum(bb[k] * xp[k:k + n] for k in range(5))
    y = xf.copy()
    at = aa[:4]
    for t in range(5, n):
        y[t] = xf[t] - (at[0] * y[t - 4] + at[1] * y[t - 3]
                        + at[2] * y[t - 2] + at[3] * y[t - 1])
    return y


def _build_mats(b32, a32):
    """Returns dict of bf16 hi/lo stationary operands (transposed for lhsT)."""
    bb = np.asarray(b32, np.float64)
    aa = np.asarray(a32, np.float64)
    M = _reference_operator(bb, aa, 3 * BL)
    L0 = M[0:BL, 0:BL]
    T0 = M[2 * BL:3 * BL, 2 * BL:3 * BL]
    T1 = M[2 * BL:3 * BL, BL:2 * BL]
    # truncation + init-transient leakage must be below fp32 noise
    leak = np.abs(M[2 * BL:3 * BL, 0:BL]).max()
    dev = max(np.abs(M[BL:2 * BL, BL:2 * BL] - T0).max(),
              np.abs(M[BL:2 * BL, 0:BL] - T1).max())
    assert leak < 1e-9 and dev < 1e-9, (leak, dev)

    out = {}
    for name, W in (("l0", L0), ("t0", T0), ("t1", T1)):
        WT = np.ascontiguousarray(W.T)          # matmul wants lhsT = W.T
        Wh = WT.astype(np.float32).astype(ml_dtypes.bfloat16)
        Wl = (WT - Wh.astype(np.float64)).astype(np.float32).astype(
            ml_dtypes.bfloat16)
        out[name + "h"] = np.ascontiguousarray(Wh)
        out[name + "l"] = np.ascontiguousarray(Wl)
    return out


def _build_program():
    nc = bacc.Bacc("TRN2", target_bir_lowering=False, debug=False)
    sig = nc.dram_tensor("sig", [2, T, C], F32, kind="ExternalInput").ap()
    wd = {n: nc.dram_tensor(n, [BL, BL], BF16, kind="ExternalInput").ap()
          for n in ("l0h", "l0l", "t0h", "t0l", "t1h", "t1l")}
    yd = nc.dram_tensor("y", [T, C], F32, kind="ExternalOutput").ap()

    # per-superbatch views: [NSB, 128part, SBW, C] over time-major DRAM
    sig_r = [sig[i].rearrange("(s b p) c -> s p b c", b=SBW, p=BL)
             for i in (0, 1)]
    y_r = yd.rearrange("(s b p) c -> s p b c", b=SBW, p=BL)

    with tile.TileContext(nc) as tc, ExitStack() as ctx:
        wpool = ctx.enter_context(tc.tile_pool(name="w", bufs=1))
        w = {}
        for n, d in wd.items():
            w[n] = wpool.tile([BL, BL], BF16, tag=n, name="w_" + n)
            nc.sync.dma_start(w[n][:], d)

        iopool = ctx.enter_context(tc.tile_pool(name="io", bufs=2))
        xpool = ctx.enter_context(tc.tile_pool(name="x", bufs=2))
        hpool = ctx.enter_context(tc.tile_pool(name="h", bufs=3))
        ypool = ctx.enter_context(tc.tile_pool(name="y", bufs=2))
        pspool = ctx.enter_context(tc.tile_pool(name="ps", bufs=8,
                                                space="PSUM"))

        def mm(ps_ap, wt, rhs_ap, start=False, stop=False):
            nc.tensor.matmul(ps_ap, w[wt][:], rhs_ap, start=start, stop=stop)

        prev_xh = prev_xl = None
        for s in range(NSB):
            re = iopool.tile([BL, SBW * C], F32, tag="re")
            im = iopool.tile([BL, SBW * C], F32, tag="im")
            nc.sync.dma_start(re[:].rearrange("p (b c) -> p b c", b=SBW),
                              sig_r[0][s])
            nc.sync.dma_start(im[:].rearrange("p (b c) -> p b c", b=SBW),
                              sig_r[1][s])

            nc.vector.tensor_mul(re[:], re[:], re[:])
            nc.vector.tensor_mul(im[:], im[:], im[:])
            x = xpool.tile([BL, SBW * C], F32, tag="x")
            nc.vector.tensor_add(x[:], re[:], im[:])

            # bf16 hi/lo split; col 0:C is a margin holding the previous
            # superbatch's last block (for the cross-block T1 term).
            xh = hpool.tile([BL, (SBW + 1) * C], BF16, tag="xh")
            xl = hpool.tile([BL, (SBW + 1) * C], BF16, tag="xl")
            nc.scalar.activation(xh[:, C:], x[:],
                                 mybir.ActivationFunctionType.Copy)
            nc.vector.tensor_sub(xl[:, C:], x[:], xh[:, C:])
            if s > 0:
                nc.vector.tensor_copy(xh[:, 0:C], prev_xh[:, SBW * C:])
                nc.vector.tensor_copy(xl[:, 0:C], prev_xl[:, SBW * C:])

            ysb = ypool.tile([BL, SBW * C], F32, tag="ysb")
            for p in range(SBW // 2):
                ps = pspool.tile([BL, 2 * C], F32, tag="ps")
                if s == 0 and p == 0:
                    # block 0: exact-init operator L0, no cross-block term
                    h0, l0_ = xh[:, C:2 * C], xl[:, C:2 * C]
                    h1, l1 = xh[:, 2 * C:3 * C], xl[:, 2 * C:3 * C]
                    ps0, ps1 = ps[:, 0:C], ps[:, C:2 * C]
                    mm(ps0, "l0h", h0, start=True)
                    mm(ps0, "l0h", l0_)
                    mm(ps0, "l0l", h0, stop=True)
                    mm(ps1, "t0h", h1, start=True)
                    mm(ps1, "t0h", l1)
                    mm(ps1, "t0l", h1)
                    mm(ps1, "t1h", h0)
                    mm(ps1, "t1h", l0_)
                    mm(ps1, "t1l", h0, stop=True)
                else:
                    cur_h = xh[:, C + p * 2 * C: C + (p + 1) * 2 * C]
                    cur_l = xl[:, C + p * 2 * C: C + (p + 1) * 2 * C]
                    sh_h = xh[:, p * 2 * C: (p + 1) * 2 * C]
                    sh_l = xl[:, p * 2 * C: (p + 1) * 2 * C]
                    mm(ps[:], "t0h", cur_h, start=True)
                    mm(ps[:], "t0h", cur_l)
                    mm(ps[:], "t0l", cur_h)
                    mm(ps[:], "t1h", sh_h)
                    mm(ps[:], "t1h", sh_l)
                    mm(ps[:], "t1l", sh_h, stop=True)

                if p % 2 == 0:
                    nc.scalar.activation(ysb[:, p * 2 * C:(p + 1) * 2 * C],
                                         ps[:],
                                         mybir.ActivationFunctionType.Copy)
                else:
                    nc.vector.tensor_copy(ysb[:, p * 2 * C:(p + 1) * 2 * C],
                                          ps[:])

            nc.sync.dma_start(y_r[s],
                              ysb[:].rearrange("p (b c) -> p b c", b=SBW))
            prev_xh, prev_xl = xh, xl

    nc.compile()
    return nc


def kernel(signal, b, a):
    global LAST_RESULTS
    signal = np.ascontiguousarray(np.asarray(signal), dtype=np.float32)
    assert signal.shape == (2, T, B), signal.shape

    wmats = _build_mats(np.asarray(b), np.asarray(a))

    if "prog" not in _program_cache:
        _program_cache["prog"] = _build_program()
    nc = _program_cache["prog"]

    in_maps = []
    for c in range(NCORES):
        sl = signal[:, :, c * C:(c + 1) * C]
        m = {"sig": np.ascontiguousarray(sl)}
        m.update(wmats)
        in_maps.append(m)

    res = run_bass_kernel_spmd(nc, in_maps, core_ids=list(range(NCORES)),
                               trace=TRACE)
    LAST_RESULTS = res

    out = np.empty((T, B), np.float32)
    for c in range(NCORES):
        out[:, c * C:(c + 1) * C] = res.results[c]["y"]
    return out


# revision 20
# speedup vs baseline: 1.6795x; 1.1523x over previous
"""Trainium2 Bass kernel for nn_LowpassDetector.

Computes: power = re^2 + im^2, 5-tap FIR (b), order-4 IIR recurrence (a)
along time, for signal [2, T=16384, B=2048] -> y [T, B].

Strategy: the FIR+IIR cascade is LTI with all poles at radius <= 0.758,
so the combined impulse response h decays below 1e-15 within 128 taps.
The whole filter is therefore exactly (to fp32) a block-Toeplitz matmul:
  y_blk[b] = T0 @ x_blk[b] + T1 @ x_blk[b-1]     (b >= 1)
  y_blk[0] = L0 @ x_blk[0]
where L0 is the exact 128x128 operator of the reference recurrence
(including its nonstandard "first 5 samples pass through" initial
condition), built on the host in float64 by running the reference on
basis vectors. Channels (2048) are sharded 256 per core across 8 cores;
time blocks of 128 map to the TensorEngine contraction dim.

Implementation notes (v3):
- Matmuls run in fp16 with the weights split hi/lo (W ~ Wh + Wl, both
  fp16, y = Wh@x + Wl@x): fp16 streams 1 col/cycle with fast weight
  loads (vs fp32's 4 cycles + slow serial LDWEIGHTS), and the split
  makes the weight rounding error ~2^-24. The only remaining error is
  the single fp16 rounding of x (~2^-12 relative, ~2e-4 on the output).
- 8 time blocks (1024 steps) are processed per superbatch: single 1MB
  input DMAs and [128, 2048] elementwise tiles amortize the ~600ns
  per-instruction issue cost that dominated v1.
- Power computation: squares run in-place on the Scalar engine, the
  add writes the fp16 matmul operand directly on Vector (no extra
  cast pass; rounding happens once, in the add).
- Two adjacent time blocks share one N=512 matmul (their channel
  columns are adjacent in the x tile free dim); PSUM tiles span two
  banks (4 blocks) so one copy per 2 pairs drains them.
"""

import sys
from contextlib import ExitStack

import numpy as np
import ml_dtypes

for _p in ("/opt/trn_rl_repo",):
    if _p not in sys.path:
        sys.path.insert(0, _p)

import concourse.bass as bass  # noqa: E402
import concourse.tile as tile  # noqa: E402
from concourse import bacc, mybir  # noqa: E402
from concourse.bass_utils import run_bass_kernel_spmd  # noqa: E402

T, B, NCORES = 16384, 2048, 8
BL = 128                # time-block size (= PE contraction dim)
NB = T // BL            # 128 time blocks
C = B // NCORES         # 256 channels per core
SBW = 8                 # time blocks per superbatch
NSB = NB // SBW         # 16 superbatches
F32 = mybir.dt.float32
F16 = mybir.dt.float16

TRACE = False           # set by test harness for NTFF profiling
LAST_RESULTS = None     # BassKernelResults of the last run (for profiling)

_program_cache = {}


def _reference_operator(bb, aa, n):
    """Exact linear operator of the reference filter on n samples (float64).

    Columns are responses to basis vectors; replicates the reference
    semantics: xf = zero-padded cross-correlation with b, first 5 outputs
    pass through, recurrence y[t] = xf[t] - sum_j a_j y[t-j] from t=5.
    """
    x = np.eye(n)
    xp = np.concatenate([np.zeros((4, n)), x], 0)
    xf = sum(bb[k] * xp[k:k + n] for k in range(5))
    y = xf.copy()
    at = aa[:4]
    for t in range(5, n):
        y[t] = xf[t] - (at[0] * y[t - 4] + at[1] * y[t - 3]
                        + at[2] * y[t - 2] + at[3] * y[t - 1])
    return y


def _build_mats(b32, a32):
    """Returns dict of bf16 hi/lo stationary operands (transposed for lhsT)."""
    bb = np.asarray(b32, np.float64)
    aa = np.asarray(a32, np.float64)
    M = _reference_operator(bb, aa, 3 * BL)
    L0 = M[0:BL, 0:BL]
    T0 = M[2 * BL:3 * BL, 2 * BL:3 * BL]
    T1 = M[2 * BL:3 * BL, BL:2 * BL]
    # truncation + init-transient leakage must be below fp32 noise
    leak = np.abs(M[2 * BL:3 * BL, 0:BL]).max()
    dev = max(np.abs(M[BL:2 * BL, BL:2 * BL] - T0).max(),
              np.abs(M[BL:2 * BL, 0:BL] - T1).max())
    assert leak < 1e-9 and dev < 1e-9, (leak, dev)

    out = {}
    for name, W in (("l0", L0), ("t0", T0), ("t1", T1)):
        WT = np.ascontiguousarray(W.T)          # matmul wants lhsT = W.T
        Wh = WT.astype(np.float16)
        Wl = (WT - Wh.astype(np.float64)).astype(np.float16)
        out[name + "h"] = np.ascontiguousarray(Wh)
        out[name + "l"] = np.ascontiguousarray(Wl)
    return out


def _build_program():
    nc = bacc.Bacc("TRN2", target_bir_lowering=False, debug=False)
    sig = nc.dram_tensor("sig", [2, T, C], F32, kind="ExternalInput").ap()
    wd = {n: nc.dram_tensor(n, [BL, BL], F16, kind="ExternalInput").ap()
          for n in ("l0h", "l0l", "t0h", "t0l", "t1h", "t1l")}
    yd = nc.dram_tensor("y", [T, C], F32, kind="ExternalOutput").ap()

    # per-superbatch views: [NSB, 128part, SBW, C] over time-major DRAM
    sig_r = [sig[i].rearrange("(s b p) c -> s p b c", b=SBW, p=BL)
             for i in (0, 1)]
    y_r = yd.rearrange("(s b p) c -> s p b c", b=SBW, p=BL)

    with tile.TileContext(nc) as tc, ExitStack() as ctx:
        wpool = ctx.enter_context(tc.tile_pool(name="w", bufs=1))
        w = {}
        for n, d in wd.items():
            w[n] = wpool.tile([BL, BL], F16, tag=n, name="w_" + n)
            nc.sync.dma_start(w[n][:], d)

        iopool = ctx.enter_context(tc.tile_pool(name="io", bufs=2))
        hpool = ctx.enter_context(tc.tile_pool(name="h", bufs=3))
        ypool = ctx.enter_context(tc.tile_pool(name="y", bufs=2))
        pspool = ctx.enter_context(tc.tile_pool(name="ps", bufs=4,
                                                space="PSUM"))

        def mm(ps_ap, wt, rhs_ap, start=False, stop=False):
            nc.tensor.matmul(ps_ap, w[wt][:], rhs_ap, start=start, stop=stop)

        prev_xh = None
        for s in range(NSB):
            re = iopool.tile([BL, SBW * C], F32, tag="re")
            im = iopool.tile([BL, SBW * C], F32, tag="im")
            nc.sync.dma_start(re[:].rearrange("p (b c) -> p b c", b=SBW),
                              sig_r[0][s])
            nc.sync.dma_start(im[:].rearrange("p (b c) -> p b c", b=SBW),
                              sig_r[1][s])

            nc.scalar.activation(re[:], re[:],
                                 mybir.ActivationFunctionType.Square)
            nc.scalar.activation(im[:], im[:],
                                 mybir.ActivationFunctionType.Square)
            # power, rounded once to fp16 by the add itself; col 0:C is a
            # margin holding the previous superbatch's last block (for the
            # cross-block T1 term).
            xh = hpool.tile([BL, (SBW + 1) * C], F16, tag="xh")
            nc.vector.tensor_add(xh[:, C:], re[:], im[:])
            if s > 0:
                nc.vector.tensor_copy(xh[:, 0:C], prev_xh[:, SBW * C:])

            ysb = ypool.tile([BL, SBW * C], F32, tag="ysb")
            for q in range(SBW // 4):        # one 2-bank psum per 2 pairs
                ps = pspool.tile([BL, 4 * C], F32, tag="ps")
                for i in range(2):
                    p = 2 * q + i
                    pp = ps[:, i * 2 * C:(i + 1) * 2 * C]
                    if s == 0 and p == 0:
                        # block 0: exact-init operator L0, no cross term
                        h0 = xh[:, C:2 * C]
                        h1 = xh[:, 2 * C:3 * C]
                        mm(pp[:, 0:C], "l0h", h0, start=True)
                        mm(pp[:, 0:C], "l0l", h0, stop=True)
                        mm(pp[:, C:2 * C], "t0h", h1, start=True)
                        mm(pp[:, C:2 * C], "t0l", h1)
                        mm(pp[:, C:2 * C], "t1h", h0)
                        mm(pp[:, C:2 * C], "t1l", h0, stop=True)
                    else:
                        cur = xh[:, C + p * 2 * C: C + (p + 1) * 2 * C]
                        sh = xh[:, p * 2 * C: (p + 1) * 2 * C]
                        mm(pp, "t0h", cur, start=True)
                        mm(pp, "t0l", cur)
                        mm(pp, "t1h", sh)
                        mm(pp, "t1l", sh, stop=True)

                dst = ysb[:, q * 4 * C:(q + 1) * 4 * C]
                if q % 2 == 0:
                    nc.scalar.activation(dst, ps[:],
                                         mybir.ActivationFunctionType.Copy)
                else:
                    nc.vector.tensor_copy(dst, ps[:])

            nc.sync.dma_start(y_r[s],
                              ysb[:].rearrange("p (b c) -> p b c", b=SBW))
            prev_xh = xh

    nc.compile()
    return nc


def kernel(signal, b, a):
    global LAST_RESULTS
    signal = np.ascontiguousarray(np.asarray(signal), dtype=np.float32)
    assert signal.shape == (2, T, B), signal.shape

    wmats = _build_mats(np.asarray(b), np.asarray(a))

    if "prog" not in _program_cache:
        _program_cache["prog"] = _build_program()
    nc = _program_cache["prog"]

    in_maps = []
    for c in range(NCORES):
        sl = signal[:, :, c * C:(c + 1) * C]
        m = {"sig": np.ascontiguousarray(sl)}
        m.update(wmats)
        in_maps.append(m)

    res = run_bass_kernel_spmd(nc, in_maps, core_ids=list(range(NCORES)),
                               trace=TRACE)
    LAST_RESULTS = res

    out = np.empty((T, B), np.float32)
    for c in range(NCORES):
        out[:, c * C:(c + 1) * C] = res.results[c]["y"]
    return out


# revision 23
# speedup vs baseline: 1.8325x; 1.0911x over previous
"""Trainium2 Bass kernel for nn_LowpassDetector.

Computes: power = re^2 + im^2, 5-tap FIR (b), order-4 IIR recurrence (a)
along time, for signal [2, T=16384, B=2048] -> y [T, B].

Strategy: the FIR+IIR cascade is LTI with all poles at radius <= 0.758,
so the combined impulse response h decays below 1e-15 within 128 taps.
The whole filter is therefore exactly (to fp32) a block-Toeplitz matmul:
  y_blk[b] = T0 @ x_blk[b] + T1 @ x_blk[b-1]     (b >= 1)
  y_blk[0] = L0 @ x_blk[0]
where L0 is the exact 128x128 operator of the reference recurrence
(including its nonstandard "first 5 samples pass through" initial
condition), built on the host in float64 by running the reference on
basis vectors. Channels (2048) are sharded 256 per core across 8 cores;
time blocks of 128 map to the TensorEngine contraction dim.

Implementation notes (v3):
- Matmuls run in fp16 with the weights split hi/lo (W ~ Wh + Wl, both
  fp16, y = Wh@x + Wl@x): fp16 streams 1 col/cycle with fast weight
  loads (vs fp32's 4 cycles + slow serial LDWEIGHTS), and the split
  makes the weight rounding error ~2^-24. The only remaining error is
  the single fp16 rounding of x (~2^-12 relative, ~2e-4 on the output).
- 8 time blocks (1024 steps) are processed per superbatch: single 1MB
  input DMAs and [128, 2048] elementwise tiles amortize the ~600ns
  per-instruction issue cost that dominated v1.
- Power computation: squares run in-place on the Scalar engine, the
  add writes the fp16 matmul operand directly on Vector (no extra
  cast pass; rounding happens once, in the add).
- Two adjacent time blocks share one N=512 matmul (their channel
  columns are adjacent in the x tile free dim); PSUM tiles span two
  banks (4 blocks) so one copy per 2 pairs drains them.
"""

import sys
from contextlib import ExitStack

import numpy as np
import ml_dtypes

for _p in ("/opt/trn_rl_repo",):
    if _p not in sys.path:
        sys.path.insert(0, _p)

import concourse.bass as bass  # noqa: E402
import concourse.tile as tile  # noqa: E402
from concourse import bacc, mybir  # noqa: E402
from concourse.bass_utils import run_bass_kernel_spmd  # noqa: E402

T, B, NCORES = 16384, 2048, 8
BL = 128                # time-block size (= PE contraction dim)
NB = T // BL            # 128 time blocks
C = B // NCORES         # 256 channels per core
SBW = 8                 # time blocks per superbatch
NSB = NB // SBW         # 16 superbatches
F32 = mybir.dt.float32
F16 = mybir.dt.float16

TRACE = False           # set by test harness for NTFF profiling
LAST_RESULTS = None     # BassKernelResults of the last run (for profiling)

_program_cache = {}


def _reference_operator(bb, aa, n):
    """Exact linear operator of the reference filter on n samples (float64).

    Columns are responses to basis vectors; replicates the reference
    semantics: xf = zero-padded cross-correlation with b, first 5 outputs
    pass through, recurrence y[t] = xf[t] - sum_j a_j y[t-j] from t=5.
    """
    x = np.eye(n)
    xp = np.concatenate([np.zeros((4, n)), x], 0)
    xf = sum(bb[k] * xp[k:k + n] for k in range(5))
    y = xf.copy()
    at = aa[:4]
    for t in range(5, n):
        y[t] = xf[t] - (at[0] * y[t - 4] + at[1] * y[t - 3]
                        + at[2] * y[t - 2] + at[3] * y[t - 1])
    return y


def _build_mats(b32, a32):
    """Returns dict of bf16 hi/lo stationary operands (transposed for lhsT)."""
    bb = np.asarray(b32, np.float64)
    aa = np.asarray(a32, np.float64)
    M = _reference_operator(bb, aa, 3 * BL)
    L0 = M[0:BL, 0:BL]
    T0 = M[2 * BL:3 * BL, 2 * BL:3 * BL]
    T1 = M[2 * BL:3 * BL, BL:2 * BL]
    # truncation + init-transient leakage must be below fp32 noise
    leak = np.abs(M[2 * BL:3 * BL, 0:BL]).max()
    dev = max(np.abs(M[BL:2 * BL, BL:2 * BL] - T0).max(),
              np.abs(M[BL:2 * BL, 0:BL] - T1).max())
    assert leak < 1e-9 and dev < 1e-9, (leak, dev)

    out = {}
    for name, W in (("l0", L0), ("t0", T0), ("t1", T1)):
        WT = np.ascontiguousarray(W.T)          # matmul wants lhsT = W.T
        Wh = WT.astype(np.float16)
        Wl = (WT - Wh.astype(np.float64)).astype(np.float16)
        out[name + "h"] = np.ascontiguousarray(Wh)
        out[name + "l"] = np.ascontiguousarray(Wl)
    return out


def _build_program():
    nc = bacc.Bacc("TRN2", target_bir_lowering=False, debug=False)
    sig = nc.dram_tensor("sig", [2, T, C], F32, kind="ExternalInput").ap()
    wd = {n: nc.dram_tensor(n, [BL, BL], F16, kind="ExternalInput").ap()
          for n in ("l0h", "l0l", "t0h", "t0l", "t1h", "t1l")}
    yd = nc.dram_tensor("y", [T, C], F32, kind="ExternalOutput").ap()

    # per-superbatch views: [NSB, 128part, SBW, C] over time-major DRAM
    sig_r = [sig[i].rearrange("(s b p) c -> s p b c", b=SBW, p=BL)
             for i in (0, 1)]
    y_r = yd.rearrange("(s b p) c -> s p b c", b=SBW, p=BL)

    with tile.TileContext(nc) as tc, ExitStack() as ctx:
        wpool = ctx.enter_context(tc.tile_pool(name="w", bufs=1))
        w = {}
        for n, d in wd.items():
            w[n] = wpool.tile([BL, BL], F16, tag=n, name="w_" + n)
            nc.sync.dma_start(w[n][:], d)

        iopool = ctx.enter_context(tc.tile_pool(name="io", bufs=3))
        hpool = ctx.enter_context(tc.tile_pool(name="h", bufs=3))
        ypool = ctx.enter_context(tc.tile_pool(name="y", bufs=3))
        pspool = ctx.enter_context(tc.tile_pool(name="ps", bufs=4,
                                                space="PSUM"))

        def mm(ps_ap, wt, rhs_ap, start=False, stop=False):
            nc.tensor.matmul(ps_ap, w[wt][:], rhs_ap, start=start, stop=stop)

        prev_xh = None
        for s in range(NSB):
            re = iopool.tile([BL, SBW * C], F32, tag="re")
            im = iopool.tile([BL, SBW * C], F32, tag="im")
            nc.sync.dma_start(re[:].rearrange("p (b c) -> p b c", b=SBW),
                              sig_r[0][s])
            nc.sync.dma_start(im[:].rearrange("p (b c) -> p b c", b=SBW),
                              sig_r[1][s])

            nc.scalar.activation(re[:], re[:],
                                 mybir.ActivationFunctionType.Square)
            nc.scalar.activation(im[:], im[:],
                                 mybir.ActivationFunctionType.Square)
            # power, rounded once to fp16 by the add itself; col 0:C is a
            # margin holding the previous superbatch's last block (for the
            # cross-block T1 term).
            xh = hpool.tile([BL, (SBW + 1) * C], F16, tag="xh")
            nc.vector.tensor_add(xh[:, C:], re[:], im[:])
            if s > 0:
                nc.vector.tensor_copy(xh[:, 0:C], prev_xh[:, SBW * C:])

            ysb = ypool.tile([BL, SBW * C], F32, tag="ysb")
            for q in range(SBW // 4):        # one 2-bank psum per 2 pairs
                ps = pspool.tile([BL, 4 * C], F32, tag="ps")
                for i in range(2):
                    p = 2 * q + i
                    pp = ps[:, i * 2 * C:(i + 1) * 2 * C]
                    if s == 0 and p == 0:
                        # block 0: exact-init operator L0, no cross term
                        h0 = xh[:, C:2 * C]
                        h1 = xh[:, 2 * C:3 * C]
                        mm(pp[:, 0:C], "l0h", h0, start=True)
                        mm(pp[:, 0:C], "l0l", h0, stop=True)
                        mm(pp[:, C:2 * C], "t0h", h1, start=True)
                        mm(pp[:, C:2 * C], "t0l", h1)
                        mm(pp[:, C:2 * C], "t1h", h0)
                        mm(pp[:, C:2 * C], "t1l", h0, stop=True)
                    else:
                        cur = xh[:, C + p * 2 * C: C + (p + 1) * 2 * C]
                        sh = xh[:, p * 2 * C: (p + 1) * 2 * C]
                        mm(pp, "t0h", cur, start=True)
                        mm(pp, "t0l", cur)
                        mm(pp, "t1h", sh)
                        mm(pp, "t1l", sh, stop=True)

                dst = ysb[:, q * 4 * C:(q + 1) * 4 * C]
                if q % 2 == 0:
                    nc.scalar.activation(dst, ps[:],
                                         mybir.ActivationFunctionType.Copy)
                else:
                    nc.vector.tensor_copy(dst, ps[:])

            nc.sync.dma_start(y_r[s],
                              ysb[:].rearrange("p (b c) -> p b c", b=SBW))
            prev_xh = xh

    nc.compile()
    return nc


def kernel(signal, b, a):
    global LAST_RESULTS
    signal = np.ascontiguousarray(np.asarray(signal), dtype=np.float32)
    assert signal.shape == (2, T, B), signal.shape

    wmats = _build_mats(np.asarray(b), np.asarray(a))

    if "prog" not in _program_cache:
        _program_cache["prog"] = _build_program()
    nc = _program_cache["prog"]

    in_maps = []
    for c in range(NCORES):
        sl = signal[:, :, c * C:(c + 1) * C]
        m = {"sig": np.ascontiguousarray(sl)}
        m.update(wmats)
        in_maps.append(m)

    res = run_bass_kernel_spmd(nc, in_maps, core_ids=list(range(NCORES)),
                               trace=TRACE)
    LAST_RESULTS = res

    out = np.empty((T, B), np.float32)
    for c in range(NCORES):
        out[:, c * C:(c + 1) * C] = res.results[c]["y"]
    return out
